# revision 19
# baseline (speedup 1.0000x reference)
"""Trainium2 Bass kernel for nn_MultiHeadAttention (B=2,S=2048,D=1024,H=16,HD=64).

Sharding: tensor-parallel over heads (2 heads/core x 8 cores).
Per core:
  Phase A: load pre-transposed, host-cast bf16 X^T (Q/K/V inputs) straight
           into SBUF (no on-device casts), project to Qt/Kt [128(2h*64), 4096]
           (transposed, bf16) and V [4096, 2x(64+ones)] (bf16).
  Phase B: per (batch, q-chunk): scores^T[keys,q] = Kt_h-tiles @ Qt_h (PE,
           bf16), exp via ACT (scale folded into Qt; no max-subtraction --
           scores are O(1) by construction), val^T[65,q] = V_ext^T @ exp_st
           (ones column -> row 64 = softmax denom), val MMs interleaved per
           double-key-tile to keep PE warm.
  Norm:    per q-chunk, immediately: copy the two denom rows into one SBUF
           tile (partition 64), reciprocal_approx_fast in place, broadcast
           to [128,512] via two K=1 ones-matmuls (no DMA), one DVE multiply
           into valT, then stage this chunk's a2a payload.
  Phase C: token ownership is interleaved (core j owns tokens
           b*2048 + half*1024 + j*128 + r), so batch-1's AllToAll splits in
           two: 1a after (1,0)/(1,1), 1b covers only the last two chunks ->
           short tail. Batch-0 uses one AllToAll, sent right after (0,3).
  Phase D: out^T[o, tok] = WpT-tiles @ concatT-tiles + bp, emitted as hook
           bursts inside later phase-B chunks; only the 1b half runs after
           the last attention chunk.
Host: pure layout prep (transposes/slices/casts) + output assembly.
"""
import sys
import numpy as np

sys.path.insert(0, "/opt/trn_rl_repo")
sys.path.insert(0, "/opt/trn_rl_repo/concourse")

import concourse.bass as bass
import concourse.tile as tile
from concourse import bacc, mybir
from concourse.bass_utils import run_bass_kernel_spmd

FP32 = mybir.dt.float32
BF16 = mybir.dt.bfloat16
AF = mybir.ActivationFunctionType
ALU = mybir.AluOpType

B, S, D, H, HD = 2, 2048, 1024, 16, 64
M = 8                 # cores
HC = H // M           # heads per core
F = HC * HD           # 128 per-core proj features
T = B * S             # 4096 tokens
TS = T // M           # 512 tokens per core (final proj)
SR = S // M           # 256 seq rows per core per batch
SCALE = HD ** -0.5
P = 128
NKT = D // P          # 8 contraction tiles
NST = S // P          # 16 key tiles per batch
NDK = NST // 2        # 8 double key tiles
NQC = S // 512        # 4 q-chunks per batch

_CACHE = {}


def build():
    nc = bacc.Bacc("TRN2", target_bir_lowering=False, debug=False, num_devices=M)

    XT = {x: nc.dram_tensor(f"{x}T", [D, T], BF16, kind="ExternalInput") for x in "qkv"}
    W2 = {x: nc.dram_tensor(f"w2{x}", [P, NKT * F], BF16, kind="ExternalInput") for x in "qkv"}
    b2q = nc.dram_tensor("b2q", [F, 1], FP32, kind="ExternalInput")
    b2k = nc.dram_tensor("b2k", [F, 1], FP32, kind="ExternalInput")
    bvb = nc.dram_tensor("bvb", [P, HC * (HD + 1)], FP32, kind="ExternalInput")
    WpT = nc.dram_tensor("WpT", [P, NKT * D], BF16, kind="ExternalInput")
    bpT = nc.dram_tensor("bpT", [P, D // P], FP32, kind="ExternalInput")
    onehot_d = nc.dram_tensor("onehot", [HC, P], FP32, kind="ExternalInput")
    outT = nc.dram_tensor("outT", [D, TS], FP32, kind="ExternalOutput")
    a2a_in0 = nc.dram_tensor("a2a_in0", [M, F, SR], BF16)
    a2a_out0 = nc.dram_tensor("a2a_out0", [M, F, SR], BF16)
    a2a_in1a = nc.dram_tensor("a2a_in1a", [M, F, SR // 2], BF16)
    a2a_out1a = nc.dram_tensor("a2a_out1a", [M, F, SR // 2], BF16)
    a2a_in1b = nc.dram_tensor("a2a_in1b", [M, F, SR // 2], BF16)
    a2a_out1b = nc.dram_tensor("a2a_out1b", [M, F, SR // 2], BF16)

    with tile.TileContext(nc) as tc:
        with (
            tc.tile_pool(name="persist", bufs=1) as persist,
            tc.tile_pool(name="xtb", bufs=6) as xtbp,
            tc.tile_pool(name="est", bufs=6) as estp,
            tc.tile_pool(name="small", bufs=6) as small,
            tc.tile_pool(name="ps_st", bufs=3, space="PSUM") as psA,   # [128,1024] = 2 banks
            tc.tile_pool(name="ps_val", bufs=2, space="PSUM") as psV,  # [128,512] = 1 bank
        ):
            # ---------- constants / weights (all direct bf16 loads) ----------
            # sync-queue order matters at startup: the first projection (k0)
            # needs only w2k + the k0 chunk + b2k, so those go first.
            w2b = {x: persist.tile([P, NKT, F], BF16, name=f"w2b_{x}") for x in "kvq"}

            def load_w2(x):
                nc.sync.dma_start(w2b[x][:], W2[x].ap().rearrange("p (kt f) -> p kt f", kt=NKT))
            b2q_sb = persist.tile([F, 1], FP32, name="b2q_sb")
            b2k_sb = persist.tile([F, 1], FP32, name="b2k_sb")
            bvb_sb = persist.tile([P, HC * (HD + 1)], FP32, name="bvb_sb")
            bpT_sb = persist.tile([P, D // P], FP32, name="bpT_sb")
            wpTb = persist.tile([P, NKT, D], BF16, name="wpTb")

            def load_wpT(half):
                nc.sync.dma_start(
                    wpTb[:, half * 4:(half + 1) * 4, :],
                    WpT.ap().rearrange("p (kt o) -> p kt o", kt=NKT)[:, half * 4:(half + 1) * 4, :],
                )
            # head selector: oh2[h, m] = (m // 64 == h); broadcasts [2,512] recips to [128,512]
            onehot2 = persist.tile([HC, P], FP32, name="onehot2")

            # warmup collective: absorb ncfw first-trigger latency while DMA streams
            warm_in = nc.dram_tensor("cc_warm_in", [M, 1, 64], BF16)
            warm_out = nc.dram_tensor("cc_warm_out", [M, 1, 64], BF16)
            wtile = small.tile([1, M * 64], BF16, name="wtile")
            nc.vector.memset(wtile[:], 1.0)
            nc.gpsimd.dma_start(warm_in.ap().rearrange("j p r -> p (j r)"), wtile[:])
            nc.gpsimd.collective_compute(
                "AllToAll", ALU.bypass, replica_groups=[list(range(M))],
                ins=[warm_in[:]], outs=[warm_out[:]])

            # persistent activations
            Qt = persist.tile([F, T], BF16, name="Qt")        # [2h*64, tok]
            Kt = persist.tile([F, T], BF16, name="Kt")
            Vx = persist.tile([P, T // P, HC * (HD + 1)], BF16, name="Vx")
            valT = persist.tile([F, T], BF16, name="valT")
            nc.vector.memset(Vx[:, :, HD:HD + 1], 1.0)        # ones columns
            nc.vector.memset(Vx[:, :, 2 * HD + 1:2 * HD + 2], 1.0)

            # ---------- phase A pieces (one 512-token chunk of one tensor) ----------
            pre = {}

            def pk(x, ch):
                t0 = ch * 512
                xtb = xtbp.tile([P, NKT, 512], BF16, name="xtb")
                nc.sync.dma_start(
                    xtb[:],
                    XT[x].ap().rearrange("(kt p) t -> p kt t", p=P)[:, :, t0:t0 + 512],
                )
                pre[f"{x}{ch}"] = xtb

            def proj_qk(x, ch, dest, sc, bias):
                t0 = ch * 512
                xtb = pre.pop(f"{x}{ch}")
                ps = psA.tile([P, 1024], FP32, name="ps_st")
                for kt in range(NKT):
                    nc.tensor.matmul(ps[:, 0:512], lhsT=w2b[x][:, kt, :], rhs=xtb[:, kt, :],
                                     start=(kt == 0), stop=(kt == NKT - 1))
                nc.vector.tensor_scalar(dest[:, t0:t0 + 512], ps[:, 0:512], sc, bias[:, 0:1],
                                        op0=ALU.mult, op1=ALU.add)

            def proj_v(ch):
                xtb = pre.pop(f"v{ch}")
                vps = psA.tile([P, 1024], FP32, name="ps_st")
                for sub in range(4):
                    for kt in range(NKT):
                        nc.tensor.matmul(vps[:, sub * F:(sub + 1) * F],
                                         lhsT=xtb[:, kt, sub * P:(sub + 1) * P],
                                         rhs=w2b["v"][:, kt, :],
                                         start=(kt == 0), stop=(kt == NKT - 1))
                for sub in range(4):
                    tt = ch * 4 + sub
                    for h in range(HC):
                        nc.vector.tensor_add(Vx[:, tt, h * 65:h * 65 + HD],
                                             vps[:, sub * F + h * HD:sub * F + (h + 1) * HD],
                                             bvb_sb[:, h * 65:h * 65 + HD])

            def pj(x, ch):
                if x == "v":
                    proj_v(ch)
                elif x == "k":
                    proj_qk("k", ch, Kt, 1.0, b2k_sb)
                else:
                    proj_qk("q", ch, Qt, SCALE, b2q_sb)

            # ---------- Phase B ----------
            # Attention is one continuous PE stream across all 8 (b,qc) phases:
            # the last double-key-tile's val MMs and the vps finish (copies, den
            # extraction, recip) are carried into the next phase's dk0 so the
            # score/exp/val pipeline never drains at a phase boundary.
            pend = {"val": None, "fin": None}
            dens = {}

            def fin_prev():
                (b, qc, vps, ceng) = pend["fin"]
                pend["fin"] = None
                q0 = b * S + qc * 512
                dstage = small.tile([P, 1024], FP32, name="dstage")
                for h in range(HC):
                    ceng_copy(ceng, dstage[HD:HD + 1, h * 512:(h + 1) * 512],
                              vps[h][HD:HD + 1, :])
                    # unnormalized val^T -> valT (bf16)
                    ceng_copy(ceng, valT[h * HD:(h + 1) * HD, q0:q0 + 512],
                              vps[h][0:HD, :])
                den = small.tile([HC, 1024], FP32, name="den")
                den_eng = nc.gpsimd if b == 0 else nc.sync
                for h in range(HC):
                    den_eng.dma_start(den[h:h + 1, 0:512],
                                      dstage[HD:HD + 1, h * 512:(h + 1) * 512])
                nc.vector.reciprocal_approx_fast(den[:, 512:1024], den[:, 0:512])
                dens[b, qc] = den

            def phase_b(b, qc, extra_work=None, copy_eng=None):
                ceng = copy_eng if copy_eng is not None else nc.vector
                q0 = b * S + qc * 512
                vps = [psV.tile([P, 512], FP32, name="ps_val") for _ in range(HC)]
                for dk in range(NDK):
                    if extra_work is not None and dk in extra_work:
                        extra_work[dk]()
                    k0 = b * S + dk * 256
                    est = []
                    for h in range(HC):
                        fo = h * HD
                        stp = psA.tile([P, 1024], FP32, name="ps_st")
                        for half in range(2):
                            nc.tensor.matmul(stp[:, half * 512:(half + 1) * 512],
                                             lhsT=Kt[fo:fo + HD, k0 + half * P:k0 + (half + 1) * P],
                                             rhs=Qt[fo:fo + HD, q0:q0 + 512],
                                             start=True, stop=True)
                        e = estp.tile([P, 1024], BF16, name="est")
                        nc.scalar.activation(e[:], stp[:], AF.Exp)
                        est.append(e)
                    # val MMs for the previous double-tile (keeps PE fed while ACT runs)
                    if pend["val"] is not None:
                        emit_val(*pend["val"])
                    if pend["fin"] is not None:
                        fin_prev()
                    pend["val"] = (b, qc, dk, est, vps)
                pend["fin"] = (b, qc, vps, ceng)

            def flush_tail():
                emit_val(*pend["val"])
                pend["val"] = None
                fin_prev()

            def norm_chunk(b, qc, den):
                q0 = b * S + qc * 512
                rbp = psA.tile([P, 1024], FP32, name="ps_st")
                nc.tensor.matmul(rbp[:, 0:512], lhsT=onehot2[:], rhs=den[:, 512:1024],
                                 start=True, stop=True)
                nc.vector.tensor_mul(valT[:, q0:q0 + 512], rbp[:, 0:512],
                                     valT[:, q0:q0 + 512])
                # stage this chunk's a2a payload:
                # chunk (b,qc) holds tokens of dest cores j=(qc%2)*4..+4, r-half qc//2
                j0 = (qc % 2) * 4
                rh = qc // 2
                src = valT.rearrange("p (bb qc jj r) -> p bb qc jj r",
                                     bb=B, qc=NQC, jj=4, r=128)[:, b, qc, :, :]
                if b == 0:
                    dst = a2a_in0.ap().rearrange("j p r -> p j r")[
                        :, j0:j0 + 4, rh * 128:(rh + 1) * 128]
                    nc.gpsimd.dma_start(dst, src)
                else:
                    buf = a2a_in1a if qc < 2 else a2a_in1b
                    dst = buf.ap().rearrange("j p r -> p j r")[:, j0:j0 + 4, :]
                    nc.sync.dma_start(dst, src)

            def ceng_copy(eng, out, in_):
                if eng is nc.scalar:
                    eng.copy(out, in_)
                else:
                    eng.tensor_copy(out, in_)

            def emit_val(b, qc, dk, est, vps):
                for h in range(HC):
                    for half in range(2):
                        kt = dk * 2 + half
                        nc.tensor.matmul(vps[h][0:HD + 1, :],
                                         lhsT=Vx[:, b * NST + kt, h * 65:(h + 1) * 65],
                                         rhs=est[h][:, half * 512:(half + 1) * 512],
                                         start=(kt == 0), stop=(kt == NST - 1))

            def a2a_send(ain, aout):
                nc.gpsimd.collective_compute(
                    "AllToAll", ALU.bypass,
                    replica_groups=[list(range(M))],
                    ins=[ain[:]], outs=[aout[:]],
                )

            # ---------- Phase D ----------
            # outT columns: (bb, rh, r) with rh in {0,1}, r in 0..127
            def phase_d_og(b, concatT, og, rh=None, out_eng=None):
                oeng = out_eng if out_eng is not None else nc.gpsimd
                rw = 256 if rh is None else 128
                ops = psA.tile([P, 1024], FP32, name="ps_st")
                for sub in range(4):
                    oc = og * 4 + sub
                    for ft in range(NKT):
                        nc.tensor.matmul(ops[:, sub * rw:(sub + 1) * rw],
                                         lhsT=wpTb[:, ft, oc * P:(oc + 1) * P],
                                         rhs=concatT[:, ft, :],
                                         start=(ft == 0), stop=(ft == NKT - 1))
                ot = small.tile([P, 1024], FP32, name="ot")
                for sub in range(4):
                    oc = og * 4 + sub
                    nc.vector.tensor_scalar_add(ot[:, sub * rw:(sub + 1) * rw],
                                                ops[:, sub * rw:(sub + 1) * rw],
                                                bpT_sb[:, oc:oc + 1])
                dstT = outT.ap().rearrange(
                    "(og oc p) (bb rh r) -> p og oc bb rh r", p=P, oc=4, bb=B, rh=2)
                if rh is None:
                    oeng.dma_start(dstT[:, og, :, b, :, :],
                                   ot.rearrange("p (oc rh r) -> p oc rh r", oc=4, rh=2))
                else:
                    oeng.dma_start(dstT[:, og, :, b, rh, :],
                                   ot[:, 0:512].rearrange("p (oc r) -> p oc r", oc=4))

            # ---------- emission ----------
            concatT0 = persist.tile([P, M, SR], BF16, name="concatT0")
            concatT1a = persist.tile([P, M, SR // 2], BF16, name="concatT1a")
            concatT1b = persist.tile([P, M, SR // 2], BF16, name="concatT1b")

            load_w2("k"); pk("k", 0)
            nc.sync.dma_start(b2k_sb[:], b2k[:])
            load_w2("v"); pk("v", 0)
            nc.sync.dma_start(bvb_sb[:], bvb[:])
            load_w2("q"); pk("q", 0)
            nc.sync.dma_start(b2q_sb[:], b2q[:])
            nc.sync.dma_start(bpT_sb[:], bpT[:])
            nc.sync.dma_start(onehot2[:], onehot_d[:])
            pj("k", 0)
            pj("v", 0)
            pj("q", 0)
            pk("k", 1); pk("v", 1); pk("k", 2); pk("v", 2)
            phase_b(0, 0, extra_work={
                2: lambda: (pj("k", 1), pj("v", 1), pk("k", 3), pk("v", 3)),
                4: lambda: (pj("k", 2), pj("v", 2), pk("q", 1), pk("k", 4)),
                6: lambda: (pj("k", 3), pj("v", 3), pk("v", 4), pj("q", 1)),
            })
            phase_b(0, 1, extra_work={
                2: lambda: norm_chunk(0, 0, dens[0, 0]),
                3: lambda: (pj("k", 4), pj("v", 4), pk("q", 2), pk("k", 5), pk("v", 5)),
                5: lambda: load_wpT(0),
                6: lambda: pj("q", 2),
            })
            phase_b(0, 2, extra_work={
                2: lambda: norm_chunk(0, 1, dens[0, 1]),
                3: lambda: (pj("k", 5), pj("v", 5), pk("q", 3), pk("k", 6), pk("v", 6)),
                5: lambda: load_wpT(1),
                6: lambda: pj("q", 3),
            })
            phase_b(0, 3, extra_work={
                2: lambda: (norm_chunk(0, 2, dens[0, 2]), pj("k", 6), pj("v", 6)),
                4: lambda: (pk("k", 7), pk("v", 7)),
                5: lambda: (pj("k", 7), pj("v", 7), pk("q", 4)),
                6: lambda: pj("q", 4),
            })
            pk("q", 5)
            phase_b(1, 0, extra_work={
                2: lambda: (norm_chunk(0, 3, dens[0, 3]),
                            a2a_send(a2a_in0, a2a_out0)),
                4: lambda: (pj("q", 5), pk("q", 6), pk("q", 7)),
            })
            phase_b(1, 1, extra_work={
                2: lambda: (norm_chunk(1, 0, dens[1, 0]), nc.sync.dma_start(
                    concatT0[:], a2a_out0.ap().rearrange("j p r -> p j r"))),
                4: lambda: pj("q", 6),
            })
            phase_b(1, 2, extra_work={
                1: lambda: (norm_chunk(1, 1, dens[1, 1]),
                            a2a_send(a2a_in1a, a2a_out1a)),
                3: lambda: phase_d_og(0, concatT0, 0),
                4: lambda: pj("q", 7),
                6: lambda: (phase_d_og(0, concatT0, 1), nc.sync.dma_start(
                    concatT1a[:], a2a_out1a.ap().rearrange("j p r -> p j r"))),
            })
            phase_b(1, 3, extra_work={
                1: lambda: norm_chunk(1, 2, dens[1, 2]),
                2: lambda: phase_d_og(1, concatT1a, 0, rh=0),
                5: lambda: phase_d_og(1, concatT1a, 1, rh=0),
            }, copy_eng=nc.scalar)
            flush_tail()
            norm_chunk(1, 3, dens[1, 3])
            a2a_send(a2a_in1b, a2a_out1b)
            nc.sync.dma_start(concatT1b[:], a2a_out1b.ap().rearrange("j p r -> p j r"))
            phase_d_og(1, concatT1b, 0, rh=1, out_eng=nc.scalar)
            phase_d_og(1, concatT1b, 1, rh=1, out_eng=nc.scalar)

    nc.compile()
    return nc


def _host_prep(inputs):
    import ml_dtypes
    bf16 = ml_dtypes.bfloat16
    f32 = np.float32
    QT = np.ascontiguousarray(inputs["Q_in"].reshape(T, D).T.astype(bf16))
    KT = np.ascontiguousarray(inputs["K_in"].reshape(T, D).T.astype(bf16))
    VT = np.ascontiguousarray(inputs["V_in"].reshape(T, D).T.astype(bf16))
    # WpT host layout: [p, kt*o] with WpTr[p, kt, o] = Wp[o, kt*128+p]
    WpTr = np.ascontiguousarray(
        inputs["Wp"].T.reshape(NKT, P, D).transpose(1, 0, 2).reshape(P, NKT * D).astype(bf16))
    bpT = np.ascontiguousarray(inputs["bp"].reshape(D // P, P).T).astype(f32, copy=False)
    oh2 = np.zeros((HC, P), f32)
    for h in range(HC):
        oh2[h, h * HD:(h + 1) * HD] = 1.0
    in_maps = []
    for c in range(M):
        sl = slice(c * HC, (c + 1) * HC)
        m = {
            "qT": QT, "kT": KT, "vT": VT, "WpT": WpTr, "bpT": bpT, "onehot": oh2,
            "b2q": (inputs["bq"][sl].reshape(F, 1) * SCALE).astype(f32),
            "b2k": inputs["bk"][sl].reshape(F, 1).astype(f32),
        }
        for key, w in (("w2q", "Wq"), ("w2k", "Wk"), ("w2v", "Wv")):
            # [D, F] -> host layout [p, kt*f]
            wd = inputs[w][sl].transpose(1, 0, 2).reshape(D, F)
            m[key] = np.ascontiguousarray(
                wd.reshape(NKT, P, F).transpose(1, 0, 2).reshape(P, NKT * F).astype(bf16))
        bvb = np.zeros((P, HC * (HD + 1)), f32)
        for h in range(HC):
            bvb[:, h * 65:h * 65 + HD] = inputs["bv"][c * HC + h][None, :]
        m["bvb"] = bvb
        in_maps.append(m)
    return in_maps


_LAST = {"exec_time_ns": None}


def kernel(**inputs):
    inputs = {k: np.asarray(v) for k, v in inputs.items()}
    if "nc" not in _CACHE:
        _CACHE["nc"] = build()
    nc = _CACHE["nc"]
    in_maps = _host_prep(inputs)
    res = run_bass_kernel_spmd(nc, in_maps, core_ids=list(range(M)),
                               trace=_LAST.get("trace", False))
    _LAST["exec_time_ns"] = res.exec_time_ns
    _LAST["res"] = res
    out = np.zeros((T, D), np.float32)
    for c in range(M):
        oT = res.results[c]["outT"].reshape(D, B, 2, 128)  # [D, bb, rh, r]
        for b in range(B):
            for rh in range(2):
                t0 = b * S + rh * 1024 + c * 128
                out[t0:t0 + 128, :] = oT[:, b, rh, :].T
    return out.reshape(B, S, D)


# revision 23
# speedup vs baseline: 1.1513x; 1.1513x over previous
"""Trainium2 Bass kernel for nn_MultiHeadAttention (B=2,S=2048,D=1024,H=16,HD=64).

Sharding: tensor-parallel over heads (2 heads/core x 8 cores).
Per core:
  Phase A: load pre-transposed, host-cast bf16 X^T (Q/K/V inputs) straight
           into SBUF (no on-device casts), project to Qt/Kt [128(2h*64), 4096]
           (transposed, bf16) and V [4096, 2x(64+ones)] (bf16).
  Phase B: per (batch, q-chunk): scores^T[keys,q] = Kt_h-tiles @ Qt_h (PE,
           bf16), exp via ACT (scale folded into Qt; no max-subtraction --
           scores are O(1) by construction), val^T[65,q] = V_ext^T @ exp_st
           (ones column -> row 64 = softmax denom), val MMs interleaved per
           double-key-tile to keep PE warm.
  Norm:    per q-chunk, immediately: copy the two denom rows into one SBUF
           tile (partition 64), reciprocal_approx_fast in place, broadcast
           to [128,512] via two K=1 ones-matmuls (no DMA), one DVE multiply
           into valT, then stage this chunk's a2a payload.
  Phase C: token ownership is interleaved (core j owns tokens
           b*2048 + half*1024 + j*128 + r), so batch-1's AllToAll splits in
           two: 1a after (1,0)/(1,1), 1b covers only the last two chunks ->
           short tail. Batch-0 uses one AllToAll, sent right after (0,3).
  Phase D: out^T[o, tok] = WpT-tiles @ concatT-tiles + bp, emitted as hook
           bursts inside later phase-B chunks; only the 1b half runs after
           the last attention chunk.
Host: pure layout prep (transposes/slices/casts) + output assembly.
"""
import sys
import numpy as np

sys.path.insert(0, "/opt/trn_rl_repo")
sys.path.insert(0, "/opt/trn_rl_repo/concourse")

import concourse.bass as bass
import concourse.tile as tile
from concourse import bacc, mybir
from concourse.bass_utils import run_bass_kernel_spmd

FP32 = mybir.dt.float32
BF16 = mybir.dt.bfloat16
AF = mybir.ActivationFunctionType
ALU = mybir.AluOpType

B, S, D, H, HD = 2, 2048, 1024, 16, 64
M = 8                 # cores
HC = H // M           # heads per core
F = HC * HD           # 128 per-core proj features
T = B * S             # 4096 tokens
TS = T // M           # 512 tokens per core (final proj)
SR = S // M           # 256 seq rows per core per batch
SCALE = HD ** -0.5
P = 128
NKT = D // P          # 8 contraction tiles
NST = S // P          # 16 key tiles per batch
NDK = NST // 2        # 8 double key tiles
NQC = S // 512        # 4 q-chunks per batch

_CACHE = {}


def build():
    nc = bacc.Bacc("TRN2", target_bir_lowering=False, debug=False, num_devices=M)

    XT = {x: nc.dram_tensor(f"{x}T", [D, T], BF16, kind="ExternalInput") for x in "qkv"}
    W2 = {x: nc.dram_tensor(f"w2{x}", [P, NKT * F], BF16, kind="ExternalInput") for x in "qkv"}
    b2q = nc.dram_tensor("b2q", [F, 1], FP32, kind="ExternalInput")
    b2k = nc.dram_tensor("b2k", [F, 1], FP32, kind="ExternalInput")
    bvb = nc.dram_tensor("bvb", [P, HC * (HD + 1)], FP32, kind="ExternalInput")
    WpT = nc.dram_tensor("WpT", [P, NKT * D], BF16, kind="ExternalInput")
    bpT = nc.dram_tensor("bpT", [P, D // P], FP32, kind="ExternalInput")
    onehot_d = nc.dram_tensor("onehot", [HC, P], FP32, kind="ExternalInput")
    outT = nc.dram_tensor("outT", [D, TS], FP32, kind="ExternalOutput")
    a2a_in0 = nc.dram_tensor("a2a_in0", [M, F, SR], BF16)
    a2a_out0 = nc.dram_tensor("a2a_out0", [M, F, SR], BF16)
    a2a_in1a = nc.dram_tensor("a2a_in1a", [M, F, SR // 2], BF16)
    a2a_out1a = nc.dram_tensor("a2a_out1a", [M, F, SR // 2], BF16)
    a2a_in1b = nc.dram_tensor("a2a_in1b", [M, F, SR // 2], BF16)
    a2a_out1b = nc.dram_tensor("a2a_out1b", [M, F, SR // 2], BF16)

    with tile.TileContext(nc) as tc:
        with (
            tc.tile_pool(name="persist", bufs=1) as persist,
            tc.tile_pool(name="xtb", bufs=6) as xtbp,
            tc.tile_pool(name="est", bufs=6) as estp,
            tc.tile_pool(name="small", bufs=6) as small,
            tc.tile_pool(name="ps_st", bufs=3, space="PSUM") as psA,   # [128,1024] = 2 banks
            tc.tile_pool(name="ps_val", bufs=2, space="PSUM") as psV,  # [128,512] = 1 bank
        ):
            # ---------- constants / weights (all direct bf16 loads) ----------
            # sync-queue order matters at startup: the first projection (k0)
            # needs only w2k + the k0 chunk + b2k, so those go first.
            w2b = {x: persist.tile([P, NKT, F], BF16, name=f"w2b_{x}") for x in "kvq"}

            def load_w2(x):
                nc.sync.dma_start(w2b[x][:], W2[x].ap().rearrange("p (kt f) -> p kt f", kt=NKT))
            b2q_sb = persist.tile([F, 1], FP32, name="b2q_sb")
            b2k_sb = persist.tile([F, 1], FP32, name="b2k_sb")
            bvb_sb = persist.tile([P, HC * (HD + 1)], FP32, name="bvb_sb")
            bpT_sb = persist.tile([P, D // P], FP32, name="bpT_sb")
            wpTb = persist.tile([P, NKT, D], BF16, name="wpTb")

            def load_wpT(half):
                nc.sync.dma_start(
                    wpTb[:, half * 4:(half + 1) * 4, :],
                    WpT.ap().rearrange("p (kt o) -> p kt o", kt=NKT)[:, half * 4:(half + 1) * 4, :],
                )
            # head selector: oh2[h, m] = (m // 64 == h); broadcasts [2,512] recips to [128,512]
            onehot2 = persist.tile([HC, P], FP32, name="onehot2")

            # warmup collective: absorb ncfw first-trigger latency while DMA streams
            warm_in = nc.dram_tensor("cc_warm_in", [M, 1, 64], BF16)
            warm_out = nc.dram_tensor("cc_warm_out", [M, 1, 64], BF16)
            wtile = small.tile([1, M * 64], BF16, name="wtile")
            nc.vector.memset(wtile[:], 1.0)
            nc.gpsimd.dma_start(warm_in.ap().rearrange("j p r -> p (j r)"), wtile[:])
            nc.gpsimd.collective_compute(
                "AllToAll", ALU.bypass, replica_groups=[list(range(M))],
                ins=[warm_in[:]], outs=[warm_out[:]])

            # persistent activations
            Qt = persist.tile([F, T], BF16, name="Qt")        # [2h*64, tok]
            Kt = persist.tile([F, T], BF16, name="Kt")
            Vx = persist.tile([P, T // P, HC * (HD + 1)], BF16, name="Vx")
            valT = persist.tile([F, T], BF16, name="valT")
            nc.vector.memset(Vx[:, :, HD:HD + 1], 1.0)        # ones columns
            nc.vector.memset(Vx[:, :, 2 * HD + 1:2 * HD + 2], 1.0)

            # ---------- phase A pieces (one 512-token chunk of one tensor) ----------
            pre = {}

            def pk(x, ch):
                t0 = ch * 512
                xtb = xtbp.tile([P, NKT, 512], BF16, name="xtb")
                nc.sync.dma_start(
                    xtb[:],
                    XT[x].ap().rearrange("(kt p) t -> p kt t", p=P)[:, :, t0:t0 + 512],
                )
                pre[f"{x}{ch}"] = xtb

            def proj_qk_parts(x, ch, dest, sc, bias):
                # two ~1.1us pieces; the PSUM accumulation group stays open
                # across them (other tiles' matmuls may interleave).
                st = {}
                t0 = ch * 512

                def pa():
                    st["xtb"] = pre.pop(f"{x}{ch}")
                    st["ps"] = psA.tile([P, 1024], FP32, name="ps_st")
                    for kt in range(4):
                        nc.tensor.matmul(st["ps"][:, 0:512], lhsT=w2b[x][:, kt, :],
                                         rhs=st["xtb"][:, kt, :],
                                         start=(kt == 0), stop=False)

                def pb2():
                    for kt in range(4, NKT):
                        nc.tensor.matmul(st["ps"][:, 0:512], lhsT=w2b[x][:, kt, :],
                                         rhs=st["xtb"][:, kt, :],
                                         start=False, stop=(kt == NKT - 1))
                    nc.vector.tensor_scalar(dest[:, t0:t0 + 512], st["ps"][:, 0:512],
                                            sc, bias[:, 0:1], op0=ALU.mult, op1=ALU.add)
                return [pa, pb2]

            def proj_v_parts(ch):
                st = {}

                def sub_mms(sub):
                    for kt in range(NKT):
                        nc.tensor.matmul(st["ps"][:, sub * F:(sub + 1) * F],
                                         lhsT=st["xtb"][:, kt, sub * P:(sub + 1) * P],
                                         rhs=w2b["v"][:, kt, :],
                                         start=(kt == 0), stop=(kt == NKT - 1))

                def pa():
                    st["xtb"] = pre.pop(f"v{ch}")
                    st["ps"] = psA.tile([P, 1024], FP32, name="ps_st")
                    sub_mms(0)
                    sub_mms(1)

                def pb2():
                    sub_mms(2)
                    sub_mms(3)
                    for sub in range(4):
                        tt = ch * 4 + sub
                        for h in range(HC):
                            nc.vector.tensor_add(Vx[:, tt, h * 65:h * 65 + HD],
                                                 st["ps"][:, sub * F + h * HD:sub * F + (h + 1) * HD],
                                                 bvb_sb[:, h * 65:h * 65 + HD])
                return [pa, pb2]

            def pj_parts(x, ch):
                if x == "v":
                    return proj_v_parts(ch)
                elif x == "k":
                    return proj_qk_parts("k", ch, Kt, 1.0, b2k_sb)
                else:
                    return proj_qk_parts("q", ch, Qt, SCALE, b2q_sb)

            def pj(x, ch):
                for f in pj_parts(x, ch):
                    f()

            # ---------- Phase B ----------
            # Attention is one continuous PE stream across all 8 (b,qc) phases:
            # the last double-key-tile's val MMs and the vps finish (copies, den
            # extraction, recip) are carried into the next phase's dk0 so the
            # score/exp/val pipeline never drains at a phase boundary.
            pend = {"val": None, "fin": None}
            dens = {}

            def fin_prev():
                (b, qc, vps, ceng) = pend["fin"]
                pend["fin"] = None
                q0 = b * S + qc * 512
                dstage = small.tile([P, 1024], FP32, name="dstage")
                for h in range(HC):
                    ceng_copy(ceng, dstage[HD:HD + 1, h * 512:(h + 1) * 512],
                              vps[h][HD:HD + 1, :])
                    # unnormalized val^T -> valT (bf16)
                    ceng_copy(ceng, valT[h * HD:(h + 1) * HD, q0:q0 + 512],
                              vps[h][0:HD, :])
                den = small.tile([HC, 1024], FP32, name="den")
                den_eng = nc.gpsimd if b == 0 else nc.sync
                for h in range(HC):
                    den_eng.dma_start(den[h:h + 1, 0:512],
                                      dstage[HD:HD + 1, h * 512:(h + 1) * 512])
                nc.vector.reciprocal_approx_fast(den[:, 512:1024], den[:, 0:512])
                dens[b, qc] = den

            def phase_b(b, qc, extra_work=None, copy_eng=None):
                ceng = copy_eng if copy_eng is not None else nc.vector
                q0 = b * S + qc * 512
                vps = [psV.tile([P, 512], FP32, name="ps_val") for _ in range(HC)]
                for dk in range(NDK):
                    if extra_work is not None and dk in extra_work:
                        extra_work[dk]()
                    k0 = b * S + dk * 256
                    est = []
                    for h in range(HC):
                        fo = h * HD
                        stp = psA.tile([P, 1024], FP32, name="ps_st")
                        for half in range(2):
                            nc.tensor.matmul(stp[:, half * 512:(half + 1) * 512],
                                             lhsT=Kt[fo:fo + HD, k0 + half * P:k0 + (half + 1) * P],
                                             rhs=Qt[fo:fo + HD, q0:q0 + 512],
                                             start=True, stop=True)
                        e = estp.tile([P, 1024], BF16, name="est")
                        nc.scalar.activation(e[:], stp[:], AF.Exp)
                        est.append(e)
                    # val MMs for the previous double-tile (keeps PE fed while ACT runs)
                    if pend["val"] is not None:
                        emit_val(*pend["val"])
                    if pend["fin"] is not None:
                        fin_prev()
                    pend["val"] = (b, qc, dk, est, vps)
                pend["fin"] = (b, qc, vps, ceng)

            def flush_tail():
                emit_val(*pend["val"])
                pend["val"] = None
                fin_prev()

            def norm_chunk(b, qc, den):
                q0 = b * S + qc * 512
                rbp = psA.tile([P, 1024], FP32, name="ps_st")
                nc.tensor.matmul(rbp[:, 0:512], lhsT=onehot2[:], rhs=den[:, 512:1024],
                                 start=True, stop=True)
                nc.vector.tensor_mul(valT[:, q0:q0 + 512], rbp[:, 0:512],
                                     valT[:, q0:q0 + 512])
                # stage this chunk's a2a payload:
                # chunk (b,qc) holds tokens of dest cores j=(qc%2)*4..+4, r-half qc//2
                j0 = (qc % 2) * 4
                rh = qc // 2
                src = valT.rearrange("p (bb qc jj r) -> p bb qc jj r",
                                     bb=B, qc=NQC, jj=4, r=128)[:, b, qc, :, :]
                if b == 0:
                    dst = a2a_in0.ap().rearrange("j p r -> p j r")[
                        :, j0:j0 + 4, rh * 128:(rh + 1) * 128]
                    nc.gpsimd.dma_start(dst, src)
                else:
                    buf = a2a_in1a if qc < 2 else a2a_in1b
                    dst = buf.ap().rearrange("j p r -> p j r")[:, j0:j0 + 4, :]
                    nc.sync.dma_start(dst, src)

            def ceng_copy(eng, out, in_):
                if eng is nc.scalar:
                    eng.copy(out, in_)
                else:
                    eng.tensor_copy(out, in_)

            def emit_val(b, qc, dk, est, vps):
                for h in range(HC):
                    for half in range(2):
                        kt = dk * 2 + half
                        nc.tensor.matmul(vps[h][0:HD + 1, :],
                                         lhsT=Vx[:, b * NST + kt, h * 65:(h + 1) * 65],
                                         rhs=est[h][:, half * 512:(half + 1) * 512],
                                         start=(kt == 0), stop=(kt == NST - 1))

            def a2a_send(ain, aout):
                nc.gpsimd.collective_compute(
                    "AllToAll", ALU.bypass,
                    replica_groups=[list(range(M))],
                    ins=[ain[:]], outs=[aout[:]],
                )

            # ---------- Phase D ----------
            # outT columns: (bb, rh, r) with rh in {0,1}, r in 0..127
            def phase_d_parts(b, concatT, og, rh=None, out_eng=None, nparts=4):
                oeng = out_eng if out_eng is not None else nc.gpsimd
                rw = 256 if rh is None else 128
                st = {}

                def sub_mms(sub):
                    oc = og * 4 + sub
                    for ft in range(NKT):
                        nc.tensor.matmul(st["ps"][:, sub * rw:(sub + 1) * rw],
                                         lhsT=wpTb[:, ft, oc * P:(oc + 1) * P],
                                         rhs=concatT[:, ft, :],
                                         start=(ft == 0), stop=(ft == NKT - 1))

                def fini():
                    ot = small.tile([P, 1024], FP32, name="ot")
                    for sub in range(4):
                        oc = og * 4 + sub
                        nc.vector.tensor_scalar_add(ot[:, sub * rw:(sub + 1) * rw],
                                                    st["ps"][:, sub * rw:(sub + 1) * rw],
                                                    bpT_sb[:, oc:oc + 1])
                    dstT = outT.ap().rearrange(
                        "(og oc p) (bb rh r) -> p og oc bb rh r", p=P, oc=4, bb=B, rh=2)
                    if rh is None:
                        oeng.dma_start(dstT[:, og, :, b, :, :],
                                       ot.rearrange("p (oc rh r) -> p oc rh r", oc=4, rh=2))
                    else:
                        oeng.dma_start(dstT[:, og, :, b, rh, :],
                                       ot[:, 0:512].rearrange("p (oc r) -> p oc r", oc=4))

                def alloc():
                    st["ps"] = psA.tile([P, 1024], FP32, name="ps_st")
                subs_per = 4 // nparts
                parts = []
                for pi in range(nparts):
                    def piece(pi=pi):
                        if pi == 0:
                            alloc()
                        for s in range(pi * subs_per, (pi + 1) * subs_per):
                            sub_mms(s)
                        if pi == nparts - 1:
                            fini()
                    parts.append(piece)
                return parts

            def phase_d_og(b, concatT, og, rh=None, out_eng=None):
                for f in phase_d_parts(b, concatT, og, rh=rh, out_eng=out_eng, nparts=1):
                    f()

            # ---------- emission ----------
            concatT0 = persist.tile([P, M, SR], BF16, name="concatT0")
            concatT1a = persist.tile([P, M, SR // 2], BF16, name="concatT1a")
            concatT1b = persist.tile([P, M, SR // 2], BF16, name="concatT1b")

            load_w2("k"); pk("k", 0)
            nc.sync.dma_start(b2k_sb[:], b2k[:])
            load_w2("v"); pk("v", 0)
            nc.sync.dma_start(bvb_sb[:], bvb[:])
            load_w2("q"); pk("q", 0)
            nc.sync.dma_start(b2q_sb[:], b2q[:])
            nc.sync.dma_start(bpT_sb[:], bpT[:])
            nc.sync.dma_start(onehot2[:], onehot_d[:])
            pj("k", 0)
            pj("v", 0)
            pj("q", 0)
            pk("k", 1); pk("v", 1); pk("k", 2); pk("v", 2)
            # each hook slot carries <= ~2us of PE work so the exp stream
            # (the attention-rate limiter) never starves on a burst
            k1 = pj_parts("k", 1); v1 = pj_parts("v", 1)
            k2 = pj_parts("k", 2); v2 = pj_parts("v", 2)
            k3 = pj_parts("k", 3); v3 = pj_parts("v", 3)
            q1 = pj_parts("q", 1)
            phase_b(0, 0, extra_work={
                1: lambda: (k1[0](), pk("k", 3)),
                2: lambda: (k1[1](), v1[0](), pk("v", 3)),
                3: lambda: (v1[1](), k2[0](), pk("q", 1)),
                4: lambda: (k2[1](), v2[0](), pk("k", 4)),
                5: lambda: (v2[1](), k3[0](), pk("v", 4)),
                6: lambda: (k3[1](), v3[0]()),
                7: lambda: (v3[1](), q1[0]()),
            })
            q1[1]()
            k4 = pj_parts("k", 4); v4 = pj_parts("v", 4)
            q2 = pj_parts("q", 2)
            phase_b(0, 1, extra_work={
                1: lambda: pk("q", 2),
                2: lambda: k4[0](),
                3: lambda: (k4[1](), pk("k", 5)),
                4: lambda: norm_chunk(0, 0, dens[0, 0]),
                5: lambda: (v4[0](), pk("v", 5)),
                6: lambda: v4[1](),
                7: lambda: q2[0](),
            })
            q2[1]()
            k5 = pj_parts("k", 5); v5 = pj_parts("v", 5)
            q3 = pj_parts("q", 3)
            phase_b(0, 2, extra_work={
                1: lambda: pk("q", 3),
                2: lambda: (k5[0](), load_wpT(0)),
                3: lambda: (k5[1](), pk("k", 6)),
                4: lambda: norm_chunk(0, 1, dens[0, 1]),
                5: lambda: (v5[0](), pk("v", 6)),
                6: lambda: (v5[1](), load_wpT(1)),
                7: lambda: q3[0](),
            })
            q3[1]()
            k6 = pj_parts("k", 6); v6 = pj_parts("v", 6)
            q4 = pj_parts("q", 4)
            phase_b(0, 3, extra_work={
                1: lambda: pk("q", 4),
                2: lambda: k6[0](),
                3: lambda: (k6[1](), pk("k", 7)),
                4: lambda: norm_chunk(0, 2, dens[0, 2]),
                5: lambda: (v6[0](), pk("v", 7)),
                6: lambda: v6[1](),
                7: lambda: q4[0](),
            })
            q4[1]()
            k7 = pj_parts("k", 7); v7 = pj_parts("v", 7)
            q5 = pj_parts("q", 5)
            phase_b(1, 0, extra_work={
                1: lambda: pk("q", 5),
                2: lambda: k7[0](),
                3: lambda: k7[1](),
                4: lambda: (norm_chunk(0, 3, dens[0, 3]),
                            a2a_send(a2a_in0, a2a_out0)),
                5: lambda: (v7[0](), pk("q", 6)),
                6: lambda: (v7[1](), pk("q", 7)),
                7: lambda: q5[0](),
            })
            q5[1]()
            q6 = pj_parts("q", 6)
            phase_b(1, 1, extra_work={
                2: lambda: q6[0](),
                3: lambda: q6[1](),
                4: lambda: (norm_chunk(1, 0, dens[1, 0]), nc.sync.dma_start(
                    concatT0[:], a2a_out0.ap().rearrange("j p r -> p j r"))),
            })
            d0og0 = phase_d_parts(0, concatT0, 0, nparts=4)
            q7 = pj_parts("q", 7)
            phase_b(1, 2, extra_work={
                2: lambda: d0og0[0](),
                3: lambda: d0og0[1](),
                4: lambda: (norm_chunk(1, 1, dens[1, 1]),
                            a2a_send(a2a_in1a, a2a_out1a)),
                5: lambda: d0og0[2](),
                6: lambda: (d0og0[3](), nc.sync.dma_start(
                    concatT1a[:], a2a_out1a.ap().rearrange("j p r -> p j r"))),
                7: lambda: q7[0](),
            })
            q7[1]()
            d0og1 = phase_d_parts(0, concatT0, 1, nparts=4)
            d1a0 = phase_d_parts(1, concatT1a, 0, rh=0, nparts=2)
            phase_b(1, 3, extra_work={
                2: lambda: d0og1[0](),
                3: lambda: d0og1[1](),
                4: lambda: (norm_chunk(1, 2, dens[1, 2]), d0og1[2]()),
                5: lambda: d0og1[3](),
                6: lambda: d1a0[0](),
                7: lambda: d1a0[1](),
            }, copy_eng=nc.scalar)
            flush_tail()
            norm_chunk(1, 3, dens[1, 3])
            a2a_send(a2a_in1b, a2a_out1b)
            # d(1a) og1 fills the PE while the final AllToAll is in flight
            phase_d_og(1, concatT1a, 1, rh=0)
            nc.sync.dma_start(concatT1b[:], a2a_out1b.ap().rearrange("j p r -> p j r"))
            phase_d_og(1, concatT1b, 0, rh=1, out_eng=nc.scalar)
            phase_d_og(1, concatT1b, 1, rh=1, out_eng=nc.scalar)

    nc.compile()
    return nc


def _host_prep(inputs):
    import ml_dtypes
    bf16 = ml_dtypes.bfloat16
    f32 = np.float32
    QT = np.ascontiguousarray(inputs["Q_in"].reshape(T, D).T.astype(bf16))
    KT = np.ascontiguousarray(inputs["K_in"].reshape(T, D).T.astype(bf16))
    VT = np.ascontiguousarray(inputs["V_in"].reshape(T, D).T.astype(bf16))
    # WpT host layout: [p, kt*o] with WpTr[p, kt, o] = Wp[o, kt*128+p]
    WpTr = np.ascontiguousarray(
        inputs["Wp"].T.reshape(NKT, P, D).transpose(1, 0, 2).reshape(P, NKT * D).astype(bf16))
    bpT = np.ascontiguousarray(inputs["bp"].reshape(D // P, P).T).astype(f32, copy=False)
    oh2 = np.zeros((HC, P), f32)
    for h in range(HC):
        oh2[h, h * HD:(h + 1) * HD] = 1.0
    in_maps = []
    for c in range(M):
        sl = slice(c * HC, (c + 1) * HC)
        m = {
            "qT": QT, "kT": KT, "vT": VT, "WpT": WpTr, "bpT": bpT, "onehot": oh2,
            "b2q": (inputs["bq"][sl].reshape(F, 1) * SCALE).astype(f32),
            "b2k": inputs["bk"][sl].reshape(F, 1).astype(f32),
        }
        for key, w in (("w2q", "Wq"), ("w2k", "Wk"), ("w2v", "Wv")):
            # [D, F] -> host layout [p, kt*f]
            wd = inputs[w][sl].transpose(1, 0, 2).reshape(D, F)
            m[key] = np.ascontiguousarray(
                wd.reshape(NKT, P, F).transpose(1, 0, 2).reshape(P, NKT * F).astype(bf16))
        bvb = np.zeros((P, HC * (HD + 1)), f32)
        for h in range(HC):
            bvb[:, h * 65:h * 65 + HD] = inputs["bv"][c * HC + h][None, :]
        m["bvb"] = bvb
        in_maps.append(m)
    return in_maps


_LAST = {"exec_time_ns": None}


def kernel(**inputs):
    inputs = {k: np.asarray(v) for k, v in inputs.items()}
    if "nc" not in _CACHE:
        _CACHE["nc"] = build()
    nc = _CACHE["nc"]
    in_maps = _host_prep(inputs)
    res = run_bass_kernel_spmd(nc, in_maps, core_ids=list(range(M)),
                               trace=_LAST.get("trace", False))
    _LAST["exec_time_ns"] = res.exec_time_ns
    _LAST["res"] = res
    out = np.zeros((T, D), np.float32)
    for c in range(M):
        oT = res.results[c]["outT"].reshape(D, B, 2, 128)  # [D, bb, rh, r]
        for b in range(B):
            for rh in range(2):
                t0 = b * S + rh * 1024 + c * 128
                out[t0:t0 + 128, :] = oT[:, b, rh, :].T
    return out.reshape(B, S, D)


# revision 28
# speedup vs baseline: 1.1554x; 1.0036x over previous
"""Trainium2 Bass kernel for nn_MultiHeadAttention (B=2,S=2048,D=1024,H=16,HD=64).

Sharding: tensor-parallel over heads (2 heads/core x 8 cores).
Per core:
  Phase A: load pre-transposed, host-cast bf16 X^T (Q/K/V inputs) straight
           into SBUF (no on-device casts), project to Qt/Kt [128(2h*64), 4096]
           (transposed, bf16) and V [4096, 2x(64+ones)] (bf16).
  Phase B: per (batch, q-chunk): scores^T[keys,q] = Kt_h-tiles @ Qt_h (PE,
           bf16), exp via ACT (scale folded into Qt; no max-subtraction --
           scores are O(1) by construction), val^T[65,q] = V_ext^T @ exp_st
           (ones column -> row 64 = softmax denom), val MMs interleaved per
           double-key-tile to keep PE warm.
  Norm:    per q-chunk, immediately: copy the two denom rows into one SBUF
           tile (partition 64), reciprocal_approx_fast in place, broadcast
           to [128,512] via two K=1 ones-matmuls (no DMA), one DVE multiply
           into valT, then stage this chunk's a2a payload.
  Phase C: token ownership is interleaved (core j owns tokens
           b*2048 + half*1024 + j*128 + r), so batch-1's AllToAll splits in
           two: 1a after (1,0)/(1,1), 1b covers only the last two chunks ->
           short tail. Batch-0 uses one AllToAll, sent right after (0,3).
  Phase D: out^T[o, tok] = WpT-tiles @ concatT-tiles + bp, emitted as hook
           bursts inside later phase-B chunks; only the 1b half runs after
           the last attention chunk.
Host: pure layout prep (transposes/slices/casts) + output assembly.
"""
import sys
import numpy as np

sys.path.insert(0, "/opt/trn_rl_repo")
sys.path.insert(0, "/opt/trn_rl_repo/concourse")

import concourse.bass as bass
import concourse.tile as tile
from concourse import bacc, mybir
from concourse.bass_utils import run_bass_kernel_spmd

FP32 = mybir.dt.float32
BF16 = mybir.dt.bfloat16
FP8 = mybir.dt.float8e4
DROW = mybir.MatmulPerfMode.DoubleRow
AF = mybir.ActivationFunctionType
ALU = mybir.AluOpType

B, S, D, H, HD = 2, 2048, 1024, 16, 64
M = 8                 # cores
HC = H // M           # heads per core
F = HC * HD           # 128 per-core proj features
T = B * S             # 4096 tokens
TS = T // M           # 512 tokens per core (final proj)
SR = S // M           # 256 seq rows per core per batch
SCALE = HD ** -0.5
P = 128
NKT = D // P          # 8 contraction tiles
NST = S // P          # 16 key tiles per batch
NDK = NST // 2        # 8 double key tiles
NQC = S // 512        # 4 q-chunks per batch

_CACHE = {}


def build():
    nc = bacc.Bacc("TRN2", target_bir_lowering=False, debug=False, num_devices=M)

    XT = {x: nc.dram_tensor(f"{x}T", [D, T], BF16, kind="ExternalInput") for x in "qkv"}
    W2 = {x: nc.dram_tensor(f"w2{x}", [P, NKT * F], BF16, kind="ExternalInput") for x in "qkv"}
    b2q = nc.dram_tensor("b2q", [F, 1], FP32, kind="ExternalInput")
    b2k = nc.dram_tensor("b2k", [F, 1], FP32, kind="ExternalInput")
    bvb = nc.dram_tensor("bvb", [P, HC * (HD + 1)], FP32, kind="ExternalInput")
    WpT = nc.dram_tensor("WpT", [P, NKT * D], BF16, kind="ExternalInput")
    bpT = nc.dram_tensor("bpT", [P, D // P], FP32, kind="ExternalInput")
    onehot_d = nc.dram_tensor("onehot", [HC, P], FP32, kind="ExternalInput")
    outT = nc.dram_tensor("outT", [D, TS], FP32, kind="ExternalOutput")
    a2a_in0 = nc.dram_tensor("a2a_in0", [M, F, SR], BF16)
    a2a_out0 = nc.dram_tensor("a2a_out0", [M, F, SR], BF16)
    a2a_in1a = nc.dram_tensor("a2a_in1a", [M, F, SR // 2], BF16)
    a2a_out1a = nc.dram_tensor("a2a_out1a", [M, F, SR // 2], BF16)
    a2a_in1b = nc.dram_tensor("a2a_in1b", [M, F, SR // 2], BF16)
    a2a_out1b = nc.dram_tensor("a2a_out1b", [M, F, SR // 2], BF16)

    with tile.TileContext(nc) as tc:
        with (
            tc.tile_pool(name="persist", bufs=1) as persist,
            tc.tile_pool(name="xtb", bufs=6) as xtbp,
            tc.tile_pool(name="est", bufs=6) as estp,
            tc.tile_pool(name="small", bufs=6) as small,
            tc.tile_pool(name="ps_st", bufs=3, space="PSUM") as psA,   # [128,1024] = 2 banks
            tc.tile_pool(name="ps_val", bufs=2, space="PSUM") as psV,  # [128,512] = 1 bank
        ):
            # ---------- constants / weights (all direct bf16 loads) ----------
            # sync-queue order matters at startup: the first projection (k0)
            # needs only w2k + the k0 chunk + b2k, so those go first.
            w2b = {x: persist.tile([P, NKT, F], BF16, name=f"w2b_{x}") for x in "kvq"}

            def load_w2(x):
                nc.sync.dma_start(w2b[x][:], W2[x].ap().rearrange("p (kt f) -> p kt f", kt=NKT))
            b2q_sb = persist.tile([F, 1], FP32, name="b2q_sb")
            b2k_sb = persist.tile([F, 1], FP32, name="b2k_sb")
            bvb_sb = persist.tile([P, HC * (HD + 1)], FP32, name="bvb_sb")
            bpT_sb = persist.tile([P, D // P], FP32, name="bpT_sb")
            wpTb = persist.tile([P, NKT, D], BF16, name="wpTb")

            def load_wpT(half):
                nc.sync.dma_start(
                    wpTb[:, half * 4:(half + 1) * 4, :],
                    WpT.ap().rearrange("p (kt o) -> p kt o", kt=NKT)[:, half * 4:(half + 1) * 4, :],
                )
            # head selector: oh2[h, m] = (m // 64 == h); broadcasts [2,512] recips to [128,512]
            onehot2 = persist.tile([HC, P], FP32, name="onehot2")

            # warmup collective: absorb ncfw first-trigger latency while DMA streams
            warm_in = nc.dram_tensor("cc_warm_in", [M, 1, 64], BF16)
            warm_out = nc.dram_tensor("cc_warm_out", [M, 1, 64], BF16)
            wtile = small.tile([1, M * 64], BF16, name="wtile")
            nc.vector.memset(wtile[:], 1.0)
            nc.gpsimd.dma_start(warm_in.ap().rearrange("j p r -> p (j r)"), wtile[:])
            nc.gpsimd.collective_compute(
                "AllToAll", ALU.bypass, replica_groups=[list(range(M))],
                ins=[warm_in[:]], outs=[warm_out[:]])

            # persistent activations
            Qt = persist.tile([F, T], BF16, name="Qt")        # [2h*64, tok]
            Kt = persist.tile([F, T], BF16, name="Kt")
            # V in fp8 with key-tile pairs packed for DoubleRow: 80-byte pair
            # stride (16B-aligned), col 64 = ones (softmax denominator trick)
            Vx = persist.tile([P, T // P // 2, HC, 2, 80], FP8, name="Vx")
            valT = persist.tile([F, T], BF16, name="valT")
            nc.vector.memset(Vx[:, :, :, :, HD:HD + 1], 1.0)  # ones columns

            # ---------- phase A pieces (one 512-token chunk of one tensor) ----------
            pre = {}

            def pk(x, ch):
                t0 = ch * 512
                xtb = xtbp.tile([P, NKT, 512], BF16, name="xtb")
                nc.sync.dma_start(
                    xtb[:],
                    XT[x].ap().rearrange("(kt p) t -> p kt t", p=P)[:, :, t0:t0 + 512],
                )
                pre[f"{x}{ch}"] = xtb

            def proj_qk_parts(x, ch, dest, sc, bias):
                # two ~1.1us pieces; the PSUM accumulation group stays open
                # across them (other tiles' matmuls may interleave).
                st = {}
                t0 = ch * 512

                def pa():
                    st["xtb"] = pre.pop(f"{x}{ch}")
                    st["ps"] = psA.tile([P, 1024], FP32, name="ps_st")
                    for kt in range(4):
                        nc.tensor.matmul(st["ps"][:, 0:512], lhsT=w2b[x][:, kt, :],
                                         rhs=st["xtb"][:, kt, :],
                                         start=(kt == 0), stop=False)

                def pb2():
                    for kt in range(4, NKT):
                        nc.tensor.matmul(st["ps"][:, 0:512], lhsT=w2b[x][:, kt, :],
                                         rhs=st["xtb"][:, kt, :],
                                         start=False, stop=(kt == NKT - 1))
                    nc.vector.tensor_scalar(dest[:, t0:t0 + 512], st["ps"][:, 0:512],
                                            sc, bias[:, 0:1], op0=ALU.mult, op1=ALU.add)
                return [pa, pb2]

            def proj_v_parts(ch):
                st = {}

                def sub_mms(sub):
                    for kt in range(NKT):
                        nc.tensor.matmul(st["ps"][:, sub * F:(sub + 1) * F],
                                         lhsT=st["xtb"][:, kt, sub * P:(sub + 1) * P],
                                         rhs=w2b["v"][:, kt, :],
                                         start=(kt == 0), stop=(kt == NKT - 1))

                def pa():
                    st["xtb"] = pre.pop(f"v{ch}")
                    st["ps"] = psA.tile([P, 1024], FP32, name="ps_st")
                    sub_mms(0)
                    sub_mms(1)

                def pb2():
                    sub_mms(2)
                    sub_mms(3)
                    for sub in range(4):
                        tt = ch * 4 + sub
                        for h in range(HC):
                            nc.vector.tensor_add(Vx[:, tt // 2, h, tt % 2, 0:HD],
                                                 st["ps"][:, sub * F + h * HD:sub * F + (h + 1) * HD],
                                                 bvb_sb[:, h * 65:h * 65 + HD])
                return [pa, pb2]

            def pj_parts(x, ch):
                if x == "v":
                    return proj_v_parts(ch)
                elif x == "k":
                    return proj_qk_parts("k", ch, Kt, 1.0, b2k_sb)
                else:
                    return proj_qk_parts("q", ch, Qt, SCALE, b2q_sb)

            def pj(x, ch):
                for f in pj_parts(x, ch):
                    f()

            # ---------- Phase B ----------
            # Attention is one continuous PE stream across all 8 (b,qc) phases:
            # the last double-key-tile's val MMs and the vps finish (copies, den
            # extraction, recip) are carried into the next phase's dk0 so the
            # score/exp/val pipeline never drains at a phase boundary.
            pend = {"val": None, "fin": None}
            dens = {}

            def fin_prev():
                (b, qc, vps, ceng) = pend["fin"]
                pend["fin"] = None
                q0 = b * S + qc * 512
                dstage = small.tile([P, 1024], FP32, name="dstage")
                for h in range(HC):
                    ceng_copy(ceng, dstage[HD:HD + 1, h * 512:(h + 1) * 512],
                              vps[h][HD:HD + 1, :])
                    # unnormalized val^T -> valT (bf16)
                    ceng_copy(ceng, valT[h * HD:(h + 1) * HD, q0:q0 + 512],
                              vps[h][0:HD, :])
                den = small.tile([HC, 1024], FP32, name="den")
                den_eng = nc.gpsimd if b == 0 else nc.sync
                for h in range(HC):
                    den_eng.dma_start(den[h:h + 1, 0:512],
                                      dstage[HD:HD + 1, h * 512:(h + 1) * 512])
                nc.vector.reciprocal_approx_fast(den[:, 512:1024], den[:, 0:512])
                dens[b, qc] = den

            def phase_b(b, qc, extra_work=None, copy_eng=None):
                ceng = copy_eng if copy_eng is not None else nc.vector
                q0 = b * S + qc * 512
                vps = [psV.tile([P, 512], FP32, name="ps_val") for _ in range(HC)]
                for dk in range(NDK):
                    if extra_work is not None and dk in extra_work:
                        extra_work[dk]()
                    k0 = b * S + dk * 256
                    est = []
                    for h in range(HC):
                        fo = h * HD
                        stp = psA.tile([P, 1024], FP32, name="ps_st")
                        for half in range(2):
                            nc.tensor.matmul(stp[:, half * 512:(half + 1) * 512],
                                             lhsT=Kt[fo:fo + HD, k0 + half * P:k0 + (half + 1) * P],
                                             rhs=Qt[fo:fo + HD, q0:q0 + 512],
                                             start=True, stop=True)
                        e = estp.tile([P, 2, 512], FP8, name="est")
                        nc.scalar.activation(e.rearrange("p a b -> p (a b)"), stp[:], AF.Exp)
                        est.append(e)
                    # val MMs for the previous double-tile (keeps PE fed while ACT runs)
                    if pend["val"] is not None:
                        emit_val(*pend["val"])
                    if pend["fin"] is not None:
                        fin_prev()
                    pend["val"] = (b, qc, dk, est, vps)
                pend["fin"] = (b, qc, vps, ceng)

            def flush_tail():
                emit_val(*pend["val"])
                pend["val"] = None
                fin_prev()

            def norm_chunk(b, qc, den):
                q0 = b * S + qc * 512
                rbp = psA.tile([P, 1024], FP32, name="ps_st")
                nc.tensor.matmul(rbp[:, 0:512], lhsT=onehot2[:], rhs=den[:, 512:1024],
                                 start=True, stop=True)
                nc.vector.tensor_mul(valT[:, q0:q0 + 512], rbp[:, 0:512],
                                     valT[:, q0:q0 + 512])
                # stage this chunk's a2a payload:
                # chunk (b,qc) holds tokens of dest cores j=(qc%2)*4..+4, r-half qc//2
                j0 = (qc % 2) * 4
                rh = qc // 2
                src = valT.rearrange("p (bb qc jj r) -> p bb qc jj r",
                                     bb=B, qc=NQC, jj=4, r=128)[:, b, qc, :, :]
                if b == 0:
                    dst = a2a_in0.ap().rearrange("j p r -> p j r")[
                        :, j0:j0 + 4, rh * 128:(rh + 1) * 128]
                    nc.gpsimd.dma_start(dst, src)
                else:
                    buf = a2a_in1a if qc < 2 else a2a_in1b
                    dst = buf.ap().rearrange("j p r -> p j r")[:, j0:j0 + 4, :]
                    nc.sync.dma_start(dst, src)

            def ceng_copy(eng, out, in_):
                if eng is nc.scalar:
                    eng.copy(out, in_)
                else:
                    eng.tensor_copy(out, in_)

            def emit_val(b, qc, dk, est, vps):
                # fp8 DoubleRow: both key tiles of the double-tile contract in
                # one pass (lhsT [128,2,65], rhs [128,2,512] -> [65,512])
                for h in range(HC):
                    nc.tensor.matmul(vps[h][0:HD + 1, :],
                                     lhsT=Vx[:, b * NDK + dk, h, :, 0:HD + 1],
                                     rhs=est[h][:],
                                     start=(dk == 0), stop=(dk == NDK - 1),
                                     perf_mode=DROW)

            def a2a_send(ain, aout):
                nc.gpsimd.collective_compute(
                    "AllToAll", ALU.bypass,
                    replica_groups=[list(range(M))],
                    ins=[ain[:]], outs=[aout[:]],
                )

            # ---------- Phase D ----------
            # outT columns: (bb, rh, r) with rh in {0,1}, r in 0..127
            def phase_d_parts(b, concatT, og, rh=None, out_eng=None, nparts=4):
                oeng = out_eng if out_eng is not None else nc.gpsimd
                rw = 256 if rh is None else 128
                st = {}

                def sub_mms(sub):
                    oc = og * 4 + sub
                    for ft in range(NKT):
                        nc.tensor.matmul(st["ps"][:, sub * rw:(sub + 1) * rw],
                                         lhsT=wpTb[:, ft, oc * P:(oc + 1) * P],
                                         rhs=concatT[:, ft, :],
                                         start=(ft == 0), stop=(ft == NKT - 1))

                def fini():
                    ot = small.tile([P, 1024], FP32, name="ot")
                    for sub in range(4):
                        oc = og * 4 + sub
                        nc.vector.tensor_scalar_add(ot[:, sub * rw:(sub + 1) * rw],
                                                    st["ps"][:, sub * rw:(sub + 1) * rw],
                                                    bpT_sb[:, oc:oc + 1])
                    dstT = outT.ap().rearrange(
                        "(og oc p) (bb rh r) -> p og oc bb rh r", p=P, oc=4, bb=B, rh=2)
                    if rh is None:
                        oeng.dma_start(dstT[:, og, :, b, :, :],
                                       ot.rearrange("p (oc rh r) -> p oc rh r", oc=4, rh=2))
                    else:
                        oeng.dma_start(dstT[:, og, :, b, rh, :],
                                       ot[:, 0:512].rearrange("p (oc r) -> p oc r", oc=4))

                def alloc():
                    st["ps"] = psA.tile([P, 1024], FP32, name="ps_st")
                subs_per = 4 // nparts
                parts = []
                for pi in range(nparts):
                    def piece(pi=pi):
                        if pi == 0:
                            alloc()
                        for s in range(pi * subs_per, (pi + 1) * subs_per):
                            sub_mms(s)
                        if pi == nparts - 1:
                            fini()
                    parts.append(piece)
                return parts

            def phase_d_og(b, concatT, og, rh=None, out_eng=None):
                for f in phase_d_parts(b, concatT, og, rh=rh, out_eng=out_eng, nparts=1):
                    f()

            # ---------- emission ----------
            concatT0 = persist.tile([P, M, SR], BF16, name="concatT0")
            concatT1a = persist.tile([P, M, SR // 2], BF16, name="concatT1a")
            concatT1b = persist.tile([P, M, SR // 2], BF16, name="concatT1b")

            load_w2("k"); pk("k", 0)
            nc.sync.dma_start(b2k_sb[:], b2k[:])
            load_w2("v"); pk("v", 0)
            nc.sync.dma_start(bvb_sb[:], bvb[:])
            load_w2("q"); pk("q", 0)
            nc.sync.dma_start(b2q_sb[:], b2q[:])
            nc.sync.dma_start(bpT_sb[:], bpT[:])
            nc.sync.dma_start(onehot2[:], onehot_d[:])
            pj("k", 0)
            pj("v", 0)
            pj("q", 0)
            pk("k", 1); pk("v", 1); pk("k", 2); pk("v", 2)
            # each hook slot carries <= ~2us of PE work so the exp stream
            # (the attention-rate limiter) never starves on a burst
            k1 = pj_parts("k", 1); v1 = pj_parts("v", 1)
            k2 = pj_parts("k", 2); v2 = pj_parts("v", 2)
            k3 = pj_parts("k", 3); v3 = pj_parts("v", 3)
            q1 = pj_parts("q", 1)
            phase_b(0, 0, extra_work={
                1: lambda: (k1[0](), pk("k", 3)),
                2: lambda: (k1[1](), v1[0](), pk("v", 3)),
                3: lambda: (v1[1](), k2[0](), pk("q", 1)),
                4: lambda: (k2[1](), v2[0](), pk("k", 4)),
                5: lambda: (v2[1](), k3[0](), pk("v", 4)),
                6: lambda: (k3[1](), v3[0]()),
                7: lambda: (v3[1](), q1[0]()),
            })
            q1[1]()
            k4 = pj_parts("k", 4); v4 = pj_parts("v", 4)
            q2 = pj_parts("q", 2)
            phase_b(0, 1, extra_work={
                1: lambda: pk("q", 2),
                2: lambda: k4[0](),
                3: lambda: (k4[1](), pk("k", 5)),
                4: lambda: norm_chunk(0, 0, dens[0, 0]),
                5: lambda: (v4[0](), pk("v", 5)),
                6: lambda: v4[1](),
                7: lambda: q2[0](),
            })
            q2[1]()
            k5 = pj_parts("k", 5); v5 = pj_parts("v", 5)
            q3 = pj_parts("q", 3)
            phase_b(0, 2, extra_work={
                1: lambda: pk("q", 3),
                2: lambda: (k5[0](), load_wpT(0)),
                3: lambda: (k5[1](), pk("k", 6)),
                4: lambda: norm_chunk(0, 1, dens[0, 1]),
                5: lambda: (v5[0](), pk("v", 6)),
                6: lambda: (v5[1](), load_wpT(1)),
                7: lambda: q3[0](),
            })
            q3[1]()
            k6 = pj_parts("k", 6); v6 = pj_parts("v", 6)
            q4 = pj_parts("q", 4)
            phase_b(0, 3, extra_work={
                1: lambda: pk("q", 4),
                2: lambda: k6[0](),
                3: lambda: (k6[1](), pk("k", 7)),
                4: lambda: norm_chunk(0, 2, dens[0, 2]),
                5: lambda: (v6[0](), pk("v", 7)),
                6: lambda: v6[1](),
                7: lambda: q4[0](),
            })
            q4[1]()
            k7 = pj_parts("k", 7); v7 = pj_parts("v", 7)
            q5 = pj_parts("q", 5)
            phase_b(1, 0, extra_work={
                1: lambda: pk("q", 5),
                2: lambda: k7[0](),
                3: lambda: k7[1](),
                4: lambda: (norm_chunk(0, 3, dens[0, 3]),
                            a2a_send(a2a_in0, a2a_out0)),
                5: lambda: (v7[0](), pk("q", 6)),
                6: lambda: (v7[1](), pk("q", 7)),
                7: lambda: q5[0](),
            })
            q5[1]()
            q6 = pj_parts("q", 6)
            phase_b(1, 1, extra_work={
                2: lambda: q6[0](),
                3: lambda: q6[1](),
                4: lambda: (norm_chunk(1, 0, dens[1, 0]), nc.sync.dma_start(
                    concatT0[:], a2a_out0.ap().rearrange("j p r -> p j r"))),
            })
            d0og0 = phase_d_parts(0, concatT0, 0, nparts=4)
            q7 = pj_parts("q", 7)
            phase_b(1, 2, extra_work={
                2: lambda: d0og0[0](),
                3: lambda: d0og0[1](),
                4: lambda: (norm_chunk(1, 1, dens[1, 1]),
                            a2a_send(a2a_in1a, a2a_out1a)),
                5: lambda: d0og0[2](),
                6: lambda: (d0og0[3](), nc.sync.dma_start(
                    concatT1a[:], a2a_out1a.ap().rearrange("j p r -> p j r"))),
                7: lambda: q7[0](),
            })
            q7[1]()
            d0og1 = phase_d_parts(0, concatT0, 1, nparts=4)
            d1a0 = phase_d_parts(1, concatT1a, 0, rh=0, nparts=2)
            phase_b(1, 3, extra_work={
                2: lambda: d0og1[0](),
                3: lambda: d0og1[1](),
                4: lambda: (norm_chunk(1, 2, dens[1, 2]), d0og1[2]()),
                5: lambda: d0og1[3](),
                6: lambda: d1a0[0](),
                7: lambda: d1a0[1](),
            }, copy_eng=nc.scalar)
            flush_tail()
            norm_chunk(1, 3, dens[1, 3])
            a2a_send(a2a_in1b, a2a_out1b)
            # d(1a) og1 fills the PE while the final AllToAll is in flight
            phase_d_og(1, concatT1a, 1, rh=0)
            nc.sync.dma_start(concatT1b[:], a2a_out1b.ap().rearrange("j p r -> p j r"))
            phase_d_og(1, concatT1b, 0, rh=1, out_eng=nc.scalar)
            phase_d_og(1, concatT1b, 1, rh=1, out_eng=nc.scalar)

    nc.compile()
    return nc


def _host_prep(inputs):
    import ml_dtypes
    bf16 = ml_dtypes.bfloat16
    f32 = np.float32
    QT = np.ascontiguousarray(inputs["Q_in"].reshape(T, D).T.astype(bf16))
    KT = np.ascontiguousarray(inputs["K_in"].reshape(T, D).T.astype(bf16))
    VT = np.ascontiguousarray(inputs["V_in"].reshape(T, D).T.astype(bf16))
    # WpT host layout: [p, kt*o] with WpTr[p, kt, o] = Wp[o, kt*128+p]
    WpTr = np.ascontiguousarray(
        inputs["Wp"].T.reshape(NKT, P, D).transpose(1, 0, 2).reshape(P, NKT * D).astype(bf16))
    bpT = np.ascontiguousarray(inputs["bp"].reshape(D // P, P).T).astype(f32, copy=False)
    oh2 = np.zeros((HC, P), f32)
    for h in range(HC):
        oh2[h, h * HD:(h + 1) * HD] = 1.0
    in_maps = []
    for c in range(M):
        sl = slice(c * HC, (c + 1) * HC)
        m = {
            "qT": QT, "kT": KT, "vT": VT, "WpT": WpTr, "bpT": bpT, "onehot": oh2,
            "b2q": (inputs["bq"][sl].reshape(F, 1) * SCALE).astype(f32),
            "b2k": inputs["bk"][sl].reshape(F, 1).astype(f32),
        }
        for key, w in (("w2q", "Wq"), ("w2k", "Wk"), ("w2v", "Wv")):
            # [D, F] -> host layout [p, kt*f]
            wd = inputs[w][sl].transpose(1, 0, 2).reshape(D, F)
            m[key] = np.ascontiguousarray(
                wd.reshape(NKT, P, F).transpose(1, 0, 2).reshape(P, NKT * F).astype(bf16))
        bvb = np.zeros((P, HC * (HD + 1)), f32)
        for h in range(HC):
            bvb[:, h * 65:h * 65 + HD] = inputs["bv"][c * HC + h][None, :]
        m["bvb"] = bvb
        in_maps.append(m)
    return in_maps


_LAST = {"exec_time_ns": None}


def kernel(**inputs):
    inputs = {k: np.asarray(v) for k, v in inputs.items()}
    if "nc" not in _CACHE:
        _CACHE["nc"] = build()
    nc = _CACHE["nc"]
    in_maps = _host_prep(inputs)
    res = run_bass_kernel_spmd(nc, in_maps, core_ids=list(range(M)),
                               trace=_LAST.get("trace", False))
    _LAST["exec_time_ns"] = res.exec_time_ns
    _LAST["res"] = res
    out = np.zeros((T, D), np.float32)
    for c in range(M):
        oT = res.results[c]["outT"].reshape(D, B, 2, 128)  # [D, bb, rh, r]
        for b in range(B):
            for rh in range(2):
                t0 = b * S + rh * 1024 + c * 128
                out[t0:t0 + 128, :] = oT[:, b, rh, :].T
    return out.reshape(B, S, D)
